# revision 1
# baseline (speedup 1.0000x reference)
"""DebiasedPosLossV2 contrastive loss on 8 Trainium2 NeuronCores.

Math (reference, B=4096, D=128, TEMP=0.5, TAU=0.1):
    out = concat([out_1, out_2])            # [2B, D], rows L2-normalized
    sim = exp(out @ out.T / TEMP)           # [2B, 2B]
    full_i = sum_j sim_ij
    keep_ij = (j%B != i%B) & ~(t_i == t_j)  where t = concat([target, target])
    Ng_i = sum_j keep_ij * sim_ij
    loss = mean(-log(o1/o2)),  o1 = full - .9*Ng,  o2 = full + (n*.1-.9)*Ng

Key identity: t_j == t_i whenever j%B == i%B (both columns carry the same
target), so keep_ij == (t_i != t_j) exactly and
    Ng_i = full_i - S_i,   S_i = sum_{j: t_j == t_i} sim_ij.

Sharding: every core holds the full X^T (the "all-gather" done host-side by
replication) and owns a 1024-column strip of sim. Because sim is symmetric,
column sums equal row sums, so a core computes, for its columns j:
    full_j = sum_i ez[i, j]                      (ones-row of the one-hot matmul)
    Q[c, j] = sum_i [t_i == c] ez[i, j]          (one-hot matmul over row blocks)
    S_j    = Q[t_j, j]                           (mask-multiply + ones matmul)
Targets live in [0, 100), so a 104-wide one-hot (+ ones column) suffices.
Per column-chunk of 512, the TensorEngine produces z = X^T[:,rb]ᵀ @ X^T[:,cols]
into PSUM, ScalarE does exp(2z) into fp16 SBUF, and a second matmul
accumulates the one-hot reduction over all 64 row blocks into one PSUM bank.
The host finishes with o1/o2/log/mean (float64) on the 2*8192 returned sums.
"""

import sys

if "/opt/trn_rl_repo" not in sys.path:
    sys.path.insert(0, "/opt/trn_rl_repo")

from contextlib import ExitStack

import numpy as np

import concourse.bass as bass
import concourse.mybir as mybir
import concourse.tile as tile
from concourse.bass import ds, ts
from concourse.bass_utils import run_bass_kernel_spmd

B = 4096
D = 128
TWO_B = 2 * B
TEMPERATURE = 0.5
TAU_PLUS = 0.1
N_CORES = 8
COLS_PER_CORE = TWO_B // N_CORES  # 1024
CHUNK = 512                       # psum bank width (fp32)
N_CHUNKS = COLS_PER_CORE // CHUNK  # 2
N_RB = TWO_B // 128               # 64 row blocks
NCLS = 100                        # target values in [0, 100)
# one-hot layout: col 0 = ones (-> full row of Q), cols 1..100 = classes,
# cols 101..127 = zero pad. Keeps every PSUM access partition-0 based
# (BIR verifier rejects PSUM APs starting at unaligned partitions), and the
# 128-wide weight tile enables fast weight load for the reduce matmul.
OHW = 128
G = 3                             # row blocks per exp() activation group

F16 = mybir.dt.float16
F32 = mybir.dt.float32

_PROGRAM = None
_PROGRAM_SPLIT = False


def _build_program() -> bass.Bass:
    nc = bass.Bass()

    # boot: the minimal data the first matmul group needs, as ONE descriptor:
    # [xt cols 0:128 | xtc chunk 0 | oh block 0] all fp16 [128, 768]
    boot_d = nc.declare_dram_parameter("boot", [128, 128 + CHUNK + OHW], F16, isOutput=False)
    # rest of row-block resources, one packed buffer per k-tile:
    # wk[k] = [xt cols k*1024:(k+1)*1024 | oh blocks 8k..8k+7]; for k=0 the
    # first 128 xt cols and oh block 0 live in boot instead.
    w0_d = nc.declare_dram_parameter("w0", [128, (1024 - 128) + 7 * OHW], F16, isOutput=False)
    wk_d = nc.declare_dram_parameter("wk", [7, 128, 1024 + 8 * OHW], F16, isOutput=False)
    xtc1_d = nc.declare_dram_parameter("xtc1", [D, CHUNK], F16, isOutput=False)
    cm_d = nc.declare_dram_parameter("cmask", [NCLS + 1, COLS_PER_CORE], F32, isOutput=False)
    fs_d = nc.declare_dram_parameter("fs", [1, 2 * COLS_PER_CORE], F32, isOutput=True)

    # first group of 1 row block starts the ScalarE exp pipeline earliest
    groups = [[0]] + [list(range(s, min(s + G, N_RB))) for s in range(1, N_RB, G)]

    with ExitStack() as ctx:
        tc = ctx.enter_context(tile.TileContext(nc))
        const = ctx.enter_context(tc.tile_pool(name="const", bufs=1))
        ezp = ctx.enter_context(tc.tile_pool(name="ez", bufs=4))
        mkp = ctx.enter_context(tc.tile_pool(name="mk", bufs=2))
        zp = ctx.enter_context(tc.tile_pool(name="z", bufs=2, space="PSUM"))
        qp = ctx.enter_context(tc.tile_pool(name="q", bufs=2, space="PSUM"))

        # Critical-path DMAs first, packed to minimize descriptor count (each
        # DMA descriptor costs ~600ns serially on the Sync sequencer): one
        # boot buffer gates the first group; the rest streams in during
        # compute, one packed buffer per k-tile.
        boot = const.tile([128, 128 + CHUNK + OHW], F16, tag="boot")
        nc.sync.dma_start(boot[:], boot_d[:])
        w0 = const.tile([128, (1024 - 128) + 7 * OHW], F16, tag="w0")
        nc.sync.dma_start(w0[:], w0_d[:])
        wks = [
            const.tile([128, 1024 + 8 * OHW], F16, tag=f"wk{k}", name=f"wk{k}")
            for k in range(1, 8)
        ]
        xtc1 = const.tile([D, CHUNK], F16, tag="xtc1")
        nc.sync.dma_start(xtc1[:], xtc1_d[:])
        for k in range(1, 8):
            nc.sync.dma_start(wks[k - 1][:], wk_d[k - 1])
        cm = const.tile([NCLS + 1, COLS_PER_CORE], F32, tag="cm")
        nc.sync.dma_start(cm[:], cm_d[:])

        xtc_h = [boot[:, 128 : 128 + CHUNK], xtc1[:]]

        def w1(rb):  # lhsT for the z matmul of row block rb
            if rb == 0:
                return boot[:, 0:128]
            if rb < 8:
                return w0[:, ts(rb - 1, 128)]
            return wks[rb // 8 - 1][:, ts(rb % 8, 128)]

        def w2(rb):  # lhsT for the one-hot reduce matmul of row block rb
            if rb == 0:
                return boot[:, 128 + CHUNK : 128 + CHUNK + OHW]
            if rb < 8:
                return w0[:, ds(896 + (rb - 1) * OHW, OHW)]
            return wks[rb // 8 - 1][:, ds(1024 + (rb % 8) * OHW, OHW)]

        ones = const.tile([NCLS + 1, 1], F16, tag="ones")
        nc.gpsimd.memset(ones[:], 1.0)
        fs = const.tile([1, 2 * COLS_PER_CORE], F32, tag="fs")
        # DVE touches cm early so the cmask-DMA wait lands on this cheap op,
        # keeping the later tensor_mul at a single sync wait (walrus limit).
        scratch = const.tile([1, 1], F32, tag="scratch")
        nc.vector.tensor_copy(scratch[:], cm[0:1, 0:1])

        def emit_groups(c, q, grps):
            for grp in grps:
                gl = len(grp)
                z = zp.tile([128, G * CHUNK], F32, tag="z", name="z")
                for s, rb in enumerate(grp):
                    nc.tensor.matmul(
                        z[:, ts(s, CHUNK)],
                        lhsT=w1(rb),
                        rhs=xtc_h[c],
                        start=True,
                        stop=True,
                        skip_group_check=True,
                    )
                ez = ezp.tile([128, G * CHUNK], F16, tag="ez", name="ez")
                nc.scalar.activation(
                    ez[:, 0 : gl * CHUNK],
                    z[:, 0 : gl * CHUNK],
                    mybir.ActivationFunctionType.Exp,
                    scale=1.0 / TEMPERATURE,
                )
                for s, rb in enumerate(grp):
                    nc.tensor.matmul(
                        q[0:OHW, :],
                        lhsT=w2(rb),
                        rhs=ez[:, ts(s, CHUNK)],
                        start=(rb == 0),
                        stop=(rb == N_RB - 1),
                        skip_group_check=True,
                    )

        def emit_extract(c, q):
            # S_j = Q[1 + t_j, j]: mask away all but row 1+t_j, then
            # ones-matmul (partition reduce). The mask-mult runs on DVE while
            # ScalarE copies the full row in parallel; the S psum lands in the
            # q slot this chunk just released (never in the z rotation, which
            # would stall the matmul pipeline).
            mk = mkp.tile([NCLS + 1, CHUNK], F16, tag="mk", name="mk")
            nc.vector.tensor_mul(mk[:], q[0 : NCLS + 1, :], cm[:, ts(c, CHUNK)])
            nc.vector.tensor_copy(fs[:, ds(c * CHUNK, CHUNK)], q[0:1, :])
            stile = qp.tile([1, CHUNK], F32, tag="q", name="stile")
            nc.tensor.matmul(
                stile[0:1, :],
                lhsT=ones[:],
                rhs=mk[:],
                start=True,
                stop=True,
                skip_group_check=True,
            )
            nc.vector.tensor_copy(
                fs[:, ds(COLS_PER_CORE + c * CHUNK, CHUNK)], stile[0:1, :]
            )

        # fs viewed as [half][chunk][512]: half 0 = full, half 1 = S
        fs4_d = fs_d.rearrange("a (h c n) -> a h c n", h=2, n=CHUNK)
        fs4 = fs.rearrange("a (h c n) -> a h c n", h=2, n=CHUNK)

        q0 = qp.tile([128, CHUNK], F32, tag="q", name="q0")
        emit_groups(0, q0, groups)
        q1 = qp.tile([128, CHUNK], F32, tag="q", name="q1")
        # Chunk-0's extraction is emitted after chunk-1's pipeline is primed
        # so the extract matmul doesn't stall the PE FIFO at the transition.
        emit_groups(1, q1, groups[:8])
        emit_extract(0, q0)
        # chunk-0 results ship out mid-kernel; only chunk-1's 4KB remains at
        # the end of the critical path.
        nc.gpsimd.dma_start(fs4_d[0:1, :, 0, :], fs4[0:1, :, 0, :])
        emit_groups(1, q1, groups[8:])
        emit_extract(1, q1)
        # SWDGE (gpsimd) for the tiny result DMAs: they get their own queue,
        # so each instruction carries a single sync wait (walrus limit).
        nc.gpsimd.dma_start(fs4_d[0:1, :, 1, :], fs4[0:1, :, 1, :])

    _strip_self_engine_waits(nc)
    return nc


def _split_drain_waits(nc: bass.Bass, max_waits: int = 1) -> None:
    """walrus codegen caps sync waits per instruction; the kernel-tail drain
    waits on all 13 processors. Split its wait list across a chain of
    preceding drains on the same engine (order of waits is immaterial; all
    must be satisfied before the block ends)."""
    for bb in nc.main_func.blocks:
        out = []
        for ins in bb.instructions:
            si = ins.sync_info
            waits = list(si.on_wait) if si and si.on_wait else []
            if type(ins).__name__ == "InstDrain" and len(waits) > max_waits:
                chunks = [
                    waits[i : i + max_waits] for i in range(0, len(waits), max_waits)
                ]
                for j, ch in enumerate(chunks[:-1]):
                    out.append(
                        mybir.InstDrain(
                            name=f"{ins.name}-w{j}",
                            ins=[],
                            outs=[],
                            engine=ins.engine,
                            sync_info=mybir.SyncInfo(on_wait=ch, on_update=[]),
                        )
                    )
                ins.sync_info = mybir.SyncInfo(
                    on_wait=chunks[-1], on_update=list(si.on_update or [])
                )
            out.append(ins)
        bb.instructions[:] = out


def _strip_self_engine_waits(nc: bass.Bass) -> None:
    """Drop semaphore waits an engine instruction holds on its *own* engine's
    semaphore when it also waits on another engine (walrus rejects >1 sync
    wait on compute-engine instructions). Engines execute their instruction
    streams strictly in order, so a wait on the issuing engine's own
    semaphore is always satisfied by program order and removing it cannot
    reorder any access."""
    prefix = {
        mybir.EngineType.Activation: "Activation_",
        mybir.EngineType.PE: "PE_",
        mybir.EngineType.DVE: "DVE_",
        mybir.EngineType.Pool: "Pool_",
    }
    for bb in nc.main_func.blocks:
        for ins in bb.instructions:
            si = ins.sync_info
            if not si or not si.on_wait or len(si.on_wait) < 2:
                continue
            pref = prefix.get(ins.engine)
            if pref is None:
                continue
            kept = [w for w in si.on_wait if not (w.ant_name or "").startswith(pref)]
            if len(kept) != len(si.on_wait):
                ins.sync_info = mybir.SyncInfo(
                    on_wait=kept, on_update=list(si.on_update)
                )


def _get_program(split_waits: bool = True) -> bass.Bass:
    """split_waits rewrites the tail drain for walrus codegen (1 sync wait
    per instruction); CoreSim chokes on the synthetic drains, so the sim
    path requests the unsplit program."""
    global _PROGRAM, _PROGRAM_SPLIT
    if _PROGRAM is None:
        _PROGRAM = _build_program()
        _PROGRAM_SPLIT = False
    if split_waits and not _PROGRAM_SPLIT:
        _split_drain_waits(_PROGRAM)
        _PROGRAM_SPLIT = True
    return _PROGRAM


def _prepare_in_maps(out_1, out_2, target):
    x = np.concatenate(
        [np.asarray(out_1, np.float32), np.asarray(out_2, np.float32)], axis=0
    )
    xt = np.ascontiguousarray(x.astype(np.float16).T)  # [128, 8192]
    t2 = np.concatenate([np.asarray(target), np.asarray(target)]).astype(np.int64)

    oh = np.zeros((TWO_B, OHW), np.float16)
    oh[:, 0] = 1.0  # ones column -> full_j row of Q (partition 0)
    oh[np.arange(TWO_B), 1 + t2] = 1.0
    # pack to [8, 128, 8*OHW]: [k, rl, p, c] -> [k, p, rl, c]
    ohp = (
        oh.reshape(8, 8, 128, OHW).transpose(0, 2, 1, 3).reshape(8, 128, 8 * OHW)
    )
    # packed per-k weight buffers: [xt k-slice | oh k-slice]
    xt3 = xt.reshape(128, 8, 1024)
    w0 = np.ascontiguousarray(
        np.concatenate([xt3[:, 0, 128:], ohp[0][:, OHW:]], axis=1)
    )
    wk = np.ascontiguousarray(
        np.concatenate([xt3.transpose(1, 0, 2)[1:], ohp[1:]], axis=2)
    )

    in_maps = []
    for core in range(N_CORES):
        c0 = core * COLS_PER_CORE
        tcols = t2[c0 : c0 + COLS_PER_CORE]
        cmask = (
            np.arange(NCLS + 1, dtype=np.int64)[:, None] == (1 + tcols)[None, :]
        ).astype(np.float32)
        boot = np.ascontiguousarray(
            np.concatenate(
                [xt[:, 0:128], xt[:, c0 : c0 + CHUNK], ohp[0][:, 0:OHW]], axis=1
            )
        )
        in_maps.append(
            {
                "boot": boot,
                "w0": w0,
                "wk": wk,
                "xtc1": np.ascontiguousarray(xt[:, c0 + CHUNK : c0 + COLS_PER_CORE]),
                "cmask": cmask,
            }
        )
    return in_maps


def _finish(fs_per_core) -> np.ndarray:
    full = np.concatenate([np.asarray(f).reshape(-1)[:COLS_PER_CORE] for f in fs_per_core]).astype(np.float64)
    s = np.concatenate([np.asarray(f).reshape(-1)[COLS_PER_CORE:] for f in fs_per_core]).astype(np.float64)
    n = TWO_B - 2
    ng = full - s
    o1 = full - (1.0 - TAU_PLUS) * ng
    o2 = full + (n * TAU_PLUS - (1.0 - TAU_PLUS)) * ng
    loss = float(np.mean(np.log(o2) - np.log(o1)))
    return np.array(loss, dtype=np.float32)


def run(out_1, out_2, out_m, target, trace=False):
    """Run on hardware; returns (loss, exec_time_ns or None)."""
    nc = _get_program()
    in_maps = _prepare_in_maps(out_1, out_2, target)
    res = run_bass_kernel_spmd(nc, in_maps, list(range(N_CORES)), trace=trace)
    fs = [res.results[i]["fs"] for i in range(N_CORES)]
    return _finish(fs), res.exec_time_ns


def kernel(out_1, out_2, out_m, target):
    loss, _ = run(out_1, out_2, out_m, target, trace=False)
    return loss

